# revision 1
# baseline (speedup 1.0000x reference)
"""Grouped-expert SwiGLU (MoE) kernel for Trainium2, expert-parallel over 8 cores.

Per core (one expert):
    g = x @ W_gate          [T, DOUT]
    u = x @ W_down          [T, DOUT]
    h = silu(g) * u
    out = h @ W_up          [T, DIN]

Layout strategy: compute transposed activations so every weight streams in
its natural HBM layout and the output lands in natural [token, din] layout.
  phase 0: xT[k] = transpose(x)          (PE transpose, bf16 eviction)
  phase 1: hT[j] = silu(Wg[:,j].T @ xT) * (Wd[:,j].T @ xT)   [dout, tokens]
  phase 2: out[m, :] = sum_j hT[j][:, m].T @ Wu[j, :]        [token, din]
Matmuls run in bf16 (fp32 PSUM accumulation); casts ride the PSUM evictions.
"""

import numpy as np

import concourse.bacc as bacc
import concourse.mybir as mybir
from concourse.tile import TileContext
from concourse.masks import make_identity
from concourse.bass_utils import run_bass_kernel_spmd

F32 = mybir.dt.float32
BF16 = mybir.dt.bfloat16
SILU = mybir.ActivationFunctionType.Silu
SIGMOID = mybir.ActivationFunctionType.Sigmoid
COPY = mybir.ActivationFunctionType.Copy

E = 8
T, DIN, DOUT = 2048, 2048, 1408
P = 128


def build_program(t=T, din=DIN, dout=DOUT, nstrip=512, sim_safe=False):
    kc = din // P   # contraction chunks for phase 1 (din)
    jc = dout // P  # dout blocks
    mc = t // P     # token blocks
    ns1 = t // nstrip    # token strips (phase 1)
    ns2 = din // nstrip  # din strips (phase 2)

    nc = bacc.Bacc(target_bir_lowering=False, trn_type="TRN2")
    x = nc.dram_tensor("x", [t, din], F32, kind="ExternalInput")
    wg = nc.dram_tensor("gate_proj", [din, dout], F32, kind="ExternalInput")
    wd = nc.dram_tensor("down_proj", [din, dout], F32, kind="ExternalInput")
    wu = nc.dram_tensor("up_proj", [dout, din], F32, kind="ExternalInput")
    out = nc.dram_tensor("out", [t, din], F32, kind="ExternalOutput")

    with TileContext(nc) as tc:
        with tc.tile_pool(name="persist", bufs=1) as persist:
            ident = persist.tile([P, P], F32, tag="ident", name="ident")
            make_identity(nc, ident)
            xT = [persist.tile([P, t], BF16, tag=f"xT{k}", name=f"xT{k}")
                  for k in range(kc)]
            hT = [persist.tile([P, t], BF16, tag=f"hT{j}", name=f"hT{j}")
                  for j in range(jc)]
            wub = [persist.tile([P, din], BF16, tag=f"wub{j}", name=f"wub{j}")
                   for j in range(jc)]

            # ---- phase 1 (with x-prep interleaved into j == 0) ----
            # x prep for one token strip: DMA f32 strip, cast to bf16,
            # DMA-transpose (x-bar) each [128,128] block into xT[k].
            half = din // 2
            with tc.tile_pool(name="xstage", bufs=6) as xstage, \
                 tc.tile_pool(name="wstage", bufs=2) as wstage, \
                 tc.tile_pool(name="wbf", bufs=2) as wbf, \
                 tc.tile_pool(name="wustage", bufs=1) as wustage, \
                 tc.tile_pool(name="silu", bufs=3) as silu_pool, \
                 tc.tile_pool(name="tpsum", bufs=4, space="PSUM") as tpsum, \
                 tc.tile_pool(name="gpsum", bufs=2, space="PSUM") as gpsum, \
                 tc.tile_pool(name="upsum", bufs=2, space="PSUM") as upsum:

                nxchunks = min(4, din // P)
                quarter = din // nxchunks

                def prep_x_blocks(mm_lo, mm_hi):
                    for mm_ in range(mm_lo, mm_hi):
                        for hh in range(nxchunks):
                            xs = xstage.tile([P, quarter], F32, tag="xs",
                                             name="xs")
                            nc.sync.dma_start(
                                out=xs,
                                in_=x.ap()[mm_ * P:(mm_ + 1) * P,
                                           hh * quarter:(hh + 1) * quarter])
                            for kk in range(quarter // P):
                                k = hh * (quarter // P) + kk
                                pt = tpsum.tile([P, P], F32, tag="pt",
                                                name="pt")
                                nc.tensor.transpose(
                                    pt, xs[:, kk * P:(kk + 1) * P], ident)
                                dst = xT[k][:, mm_ * P:(mm_ + 1) * P]
                                if k % 2 == 0:
                                    nc.scalar.activation(dst, pt, COPY)
                                else:
                                    nc.vector.tensor_copy(out=dst, in_=pt)

                def stage_weights(j):
                    wg_bf = wbf.tile([P, din], BF16, tag="wg_bf", name="wg_bf")
                    wd_bf = wbf.tile([P, din], BF16, tag="wd_bf", name="wd_bf")
                    for w_dram, w_bf, tg in ((wg, wg_bf, "g"), (wd, wd_bf, "d")):
                        for h in range(2):
                            st = wstage.tile([P, half], F32, tag=f"wst{tg}",
                                             name=f"wst{tg}{h}")
                            src = w_dram.ap()[h * half:(h + 1) * half,
                                              j * P:(j + 1) * P] \
                                .rearrange("(c p) n -> p c n", p=P)
                            dst = st.rearrange("p (c n) -> p c n", n=P)
                            nc.sync.dma_start(out=dst, in_=src)
                            if tg == "g":
                                nc.scalar.activation(
                                    w_bf[:, h * half:(h + 1) * half], st, COPY)
                            else:
                                nc.vector.tensor_copy(
                                    out=w_bf[:, h * half:(h + 1) * half],
                                    in_=st)
                    # cast this j's Wu panel while PE runs phase-1 matmuls
                    wust = wustage.tile([P, din], F32, tag="wust", name="wust")
                    nc.sync.dma_start(out=wust, in_=wu.ap()[j * P:(j + 1) * P, :])
                    nc.vector.tensor_copy(out=wub[j], in_=wust)
                    return wg_bf, wd_bf

                bps = nstrip // P  # token blocks per strip
                prep_x_blocks(0, bps)
                for j in range(jc):
                    wg_bf, wd_bf = stage_weights(j)
                    for n in range(ns1):
                        tok = slice(n * nstrip, (n + 1) * nstrip)
                        pg = gpsum.tile([P, nstrip], F32, tag="pg", name="pg")
                        pu = upsum.tile([P, nstrip], F32, tag="pu", name="pu")
                        if j == 0 and n + 1 < ns1:
                            prep_x_blocks((n + 1) * bps, (n + 1) * bps + bps // 2)
                        for k in range(kc):
                            nc.tensor.matmul(
                                pg, lhsT=wg_bf[:, k * P:(k + 1) * P],
                                rhs=xT[k][:, tok],
                                start=(k == 0), stop=(k == kc - 1))
                        if j == 0 and n + 1 < ns1:
                            prep_x_blocks((n + 1) * bps + bps // 2,
                                          (n + 2) * bps)
                        for k in range(kc):
                            nc.tensor.matmul(
                                pu, lhsT=wd_bf[:, k * P:(k + 1) * P],
                                rhs=xT[k][:, tok],
                                start=(k == 0), stop=(k == kc - 1))
                        sl = silu_pool.tile([P, nstrip], BF16, tag="sl", name="sl")
                        if sim_safe:
                            # CoreSim has no Silu; silu(g) = g * sigmoid(g)
                            nc.scalar.activation(sl, pg, SIGMOID)
                            nc.vector.tensor_mul(out=sl, in0=sl, in1=pg)
                        else:
                            nc.scalar.activation(sl, pg, SILU)
                        nc.vector.tensor_mul(out=hT[j][:, tok], in0=sl, in1=pu)

            # ---- phase 2: out = hT.T @ Wu ----
            nstrip2 = nstrip
            ns2 = din // nstrip2
            with tc.tile_pool(name="ostage", bufs=3) as ostage, \
                 tc.tile_pool(name="opsum", bufs=3, space="PSUM") as opsum:
                for m in range(mc):
                    for n in range(ns2):
                        dsl = slice(n * nstrip2, (n + 1) * nstrip2)
                        po = opsum.tile([P, nstrip2], F32, tag="po", name="po")
                        for j in range(jc):
                            nc.tensor.matmul(
                                po, lhsT=hT[j][:, m * P:(m + 1) * P],
                                rhs=wub[j][:, dsl],
                                start=(j == 0), stop=(j == jc - 1))
                        ot = ostage.tile([P, nstrip2], F32, tag="ot", name="ot")
                        if (m * ns2 + n) % 2 == 0:
                            nc.scalar.activation(ot, po, COPY)
                        else:
                            nc.vector.tensor_copy(out=ot, in_=po)
                        nc.sync.dma_start(
                            out=out.ap()[m * P:(m + 1) * P, dsl], in_=ot)

    nc.finalize()
    return nc


_program = None


def kernel(x, gate_proj, down_proj, up_proj):
    global _program
    if _program is None:
        _program = build_program()
    in_maps = [
        {
            "x": np.ascontiguousarray(x[e], dtype=np.float32),
            "gate_proj": np.ascontiguousarray(gate_proj[e], dtype=np.float32),
            "down_proj": np.ascontiguousarray(down_proj[e], dtype=np.float32),
            "up_proj": np.ascontiguousarray(up_proj[e], dtype=np.float32),
        }
        for e in range(E)
    ]
    res = run_bass_kernel_spmd(_program, in_maps, list(range(E)))
    return np.stack([res.results[e]["out"] for e in range(E)], axis=0)



# revision 3
# speedup vs baseline: 1.0959x; 1.0959x over previous
"""Grouped-expert SwiGLU (MoE) kernel for Trainium2, expert-parallel over 8 cores.

Per core (one expert):
    g = x @ W_gate          [T, DOUT]
    u = x @ W_down          [T, DOUT]
    h = silu(g) * u
    out = h @ W_up          [T, DIN]

All inputs are pre-cast to bf16 and pre-laid-out on the host so the device
does no transposes and no input casts — the PE runs a dense LDW+MM stream at
the bf16 roofline (~216 ns per [128x128]x[128x512] matmul):
  x_t    [S1, KC, P, NS]  xT chunks: x_t[s,k,p,n] = x[s*NS+n, k*P+p]
  gate_t [JC, P, DIN]     per-j panels: gate_t[j,p,k*P+n] = Wg[k*P+p, j*P+n]
  down_t [JC, P, DIN]     same layout as gate_t
  up_t   [JC, P, DIN]     up_t[j,p,c] = Wu[j*P+p, c]
phase 1: hT[j] = silu(Wg[:,j].T @ xT) * (Wd[:,j].T @ xT)   [dout, tokens]
phase 2: out[m,:] = sum_j hT[j][:,m].T @ Wu[j,:]           [tokens, din]
Matmuls in bf16 with fp32 PSUM accumulation.
"""

import numpy as np
import ml_dtypes

import concourse.bacc as bacc
import concourse.mybir as mybir
from concourse.tile import TileContext
from concourse.bass_utils import run_bass_kernel_spmd

F32 = mybir.dt.float32
BF16 = mybir.dt.bfloat16
SILU = mybir.ActivationFunctionType.Silu
SIGMOID = mybir.ActivationFunctionType.Sigmoid
COPY = mybir.ActivationFunctionType.Copy

E = 8
T, DIN, DOUT = 2048, 2048, 1408
P = 128
NS = 512
KC = DIN // P   # 16 contraction chunks (din)
JC = DOUT // P  # 11 dout blocks
MC = T // P     # 16 token blocks
S1 = T // NS    # 4 token strips
S2 = DIN // NS  # 4 din strips


def build_program(sim_safe=False):
    nc = bacc.Bacc(target_bir_lowering=False, trn_type="TRN2")
    xt = nc.dram_tensor("x_t", [S1, KC, P, NS], BF16, kind="ExternalInput")
    wg = nc.dram_tensor("gate_t", [JC, P, DIN], BF16, kind="ExternalInput")
    wd = nc.dram_tensor("down_t", [JC, P, DIN], BF16, kind="ExternalInput")
    wu = nc.dram_tensor("up_t", [JC, P, DIN], BF16, kind="ExternalInput")
    out = nc.dram_tensor("out", [T, DIN], F32, kind="ExternalOutput")

    with TileContext(nc) as tc:
        with tc.tile_pool(name="persist", bufs=1) as persist:
            xts = [[persist.tile([P, NS], BF16, tag=f"xt{s}_{k}",
                                 name=f"xt{s}_{k}")
                    for k in range(KC)] for s in range(S1)]
            hT = [persist.tile([P, T], BF16, tag=f"hT{j}", name=f"hT{j}")
                  for j in range(JC)]
            wub = [persist.tile([P, DIN], BF16, tag=f"wub{j}", name=f"wub{j}")
                   for j in range(JC)]

            with tc.tile_pool(name="wstage", bufs=2) as wstage, \
                 tc.tile_pool(name="silu", bufs=3) as silu_pool, \
                 tc.tile_pool(name="ostage", bufs=4) as ostage, \
                 tc.tile_pool(name="p1", bufs=2, space="PSUM") as p1, \
                 tc.tile_pool(name="p2", bufs=4, space="PSUM") as p2:

                # strip-0 xT chunks first so the PE can start immediately
                for k in range(KC):
                    nc.sync.dma_start(out=xts[0][k], in_=xt.ap()[0, k])

                # ---- phase 1: hT[j] = silu(gT) * uT ----
                for j in range(JC):
                    wgp = wstage.tile([P, DIN], BF16, tag="wgp", name=f"wgp{j}")
                    wdp = wstage.tile([P, DIN], BF16, tag="wdp", name=f"wdp{j}")
                    nc.sync.dma_start(out=wgp, in_=wg.ap()[j])
                    nc.sync.dma_start(out=wdp, in_=wd.ap()[j])
                    # stage phase-2 weights during phase 1
                    nc.sync.dma_start(out=wub[j], in_=wu.ap()[j])
                    for s in range(S1):
                        if j == 0 and s + 1 < S1:
                            for k in range(KC):
                                nc.sync.dma_start(out=xts[s + 1][k],
                                                  in_=xt.ap()[s + 1, k])
                        pg = p1.tile([P, NS], F32, tag="pg", name="pg")
                        pu = p1.tile([P, NS], F32, tag="pu", name="pu")
                        for k in range(KC):
                            nc.tensor.matmul(
                                pg, lhsT=wgp[:, k * P:(k + 1) * P],
                                rhs=xts[s][k],
                                start=(k == 0), stop=(k == KC - 1))
                        for k in range(KC):
                            nc.tensor.matmul(
                                pu, lhsT=wdp[:, k * P:(k + 1) * P],
                                rhs=xts[s][k],
                                start=(k == 0), stop=(k == KC - 1))
                        sl = silu_pool.tile([P, NS], BF16, tag="sl", name="sl")
                        if sim_safe:
                            # CoreSim has no Silu; silu(g) = g * sigmoid(g)
                            nc.scalar.activation(sl, pg, SIGMOID)
                            nc.vector.tensor_mul(out=sl, in0=sl, in1=pg)
                        else:
                            nc.scalar.activation(sl, pg, SILU)
                        nc.vector.tensor_mul(out=hT[j][:, s * NS:(s + 1) * NS],
                                             in0=sl, in1=pu)

                # ---- phase 2: out = hT.T @ Wu ----
                for m in range(MC):
                    for n in range(S2):
                        dsl = slice(n * NS, (n + 1) * NS)
                        po = p2.tile([P, NS], F32, tag="po", name="po")
                        for j in range(JC):
                            nc.tensor.matmul(
                                po, lhsT=hT[j][:, m * P:(m + 1) * P],
                                rhs=wub[j][:, dsl],
                                start=(j == 0), stop=(j == JC - 1))
                        ot = ostage.tile([P, NS], F32, tag="ot", name="ot")
                        if (m * S2 + n) % 2 == 0:
                            nc.scalar.activation(ot, po, COPY)
                        else:
                            nc.vector.tensor_copy(out=ot, in_=po)
                        nc.sync.dma_start(
                            out=out.ap()[m * P:(m + 1) * P, dsl], in_=ot)

    nc.finalize()
    return nc


_BF = ml_dtypes.bfloat16


def make_in_maps(x, gate_proj, down_proj, up_proj):
    maps = []
    for e in range(E):
        xtb = x[e].T.astype(_BF)  # [DIN, T]
        xtb = np.ascontiguousarray(
            xtb.reshape(KC, P, S1, NS).transpose(2, 0, 1, 3))
        gtb = np.ascontiguousarray(
            gate_proj[e].astype(_BF).reshape(KC, P, JC, P)
            .transpose(2, 1, 0, 3)).reshape(JC, P, DIN)
        dtb = np.ascontiguousarray(
            down_proj[e].astype(_BF).reshape(KC, P, JC, P)
            .transpose(2, 1, 0, 3)).reshape(JC, P, DIN)
        utb = np.ascontiguousarray(up_proj[e].astype(_BF)).reshape(JC, P, DIN)
        maps.append({"x_t": xtb, "gate_t": gtb, "down_t": dtb, "up_t": utb})
    return maps


_program = None


def kernel(x, gate_proj, down_proj, up_proj):
    global _program
    if _program is None:
        _program = build_program()
    in_maps = make_in_maps(
        np.asarray(x, dtype=np.float32),
        np.asarray(gate_proj, dtype=np.float32),
        np.asarray(down_proj, dtype=np.float32),
        np.asarray(up_proj, dtype=np.float32),
    )
    res = run_bass_kernel_spmd(_program, in_maps, list(range(E)))
    return np.stack([res.results[e]["out"] for e in range(E)], axis=0)
